# revision 8
# baseline (speedup 1.0000x reference)
"""Trainium2 Bass kernel for HDSLinear (gumbel top-2-of-4 masked linear).

v4 strategy — 2D sharding (s, o) = (2, 4) over 8 cores:
  The kernel is DMA-bound on this platform (~105 GB/s/core effective), so
  the sharding minimizes per-core HBM bytes:
    column-parallel (1,8): x 134MB + s/n 16.8 + w 4.2 + out 16.7 = 172MB
    this (2,4):            x  67MB + s/n 33.6 + w 8.4 + out 16.8 = 126MB
  Each core handles s-half (8192 rows) x o-shard (1024 out-features).

  Device pipeline per core:
  - phase 1: mask gen from scores+gumbel (ACT 2x Ln, DVE rank-select in
    stride-1 layout via host d-permutation), masked bf16 weight transposed
    on-chip into wmt[p, k, o] (xbar DMA transpose).
  - phase 2: PE matmul chains (32 k-tiles -> one PSUM bank) per 128-row
    s-tile, split into two 512-wide o-halves so half-0 chains only wait on
    the first half of phase 1; DVE adds bias (broadcast tile) and
    downcasts to bf16; out streamed as [8192, 1024] bf16.
  - host: assemble [2, 4] grid of shards, upcast to fp32.
"""

import os
import sys
import numpy as np

for _p in ("/opt/trn_rl_repo", "/root/.axon_site/_ro/trn_rl_repo"):
    if os.path.isdir(_p) and _p not in sys.path:
        sys.path.insert(0, _p)

import concourse.bass as bass
import concourse.bacc as bacc
import concourse.mybir as mybir
from concourse import tile
from concourse.bass_utils import run_bass_kernel_spmd

F32 = mybir.dt.float32
BF16 = mybir.dt.bfloat16
BF16_NP = mybir.dt.np(BF16)
AF = mybir.ActivationFunctionType
ALU = mybir.AluOpType

B, S, D_IN, D_OUT = 8, 2048, 4096, 4096
N_CORES = 8
S_WAYS, O_WAYS = 2, 4
S_TOT = B * S                      # 16384
S_SH = S_TOT // S_WAYS             # 8192 s-rows per core
O_SH = D_OUT // O_WAYS             # 1024 out-features per core
P = 128
EPS = 1e-10

K_TILES = D_IN // P                # 32 contraction tiles
S_BLK = 512                        # s-columns per phase-2 block
N_BLK = S_SH // S_BLK              # 16 blocks
O_TILES = O_SH // P                # 8 o-tiles of 128 rows in phase 1
OH = O_SH // 2                     # 512: o-half width (one chain)
GC = 512                           # groups per phase-1 tile chunk
G = D_IN // 4                      # 1024 groups per row

LAST_EXEC_NS = None
_CACHED = {}


def _build_nc():
    nc = bacc.Bacc(None, target_bir_lowering=False)
    xd = nc.declare_dram_parameter("xd", [N_BLK * P, K_TILES * S_BLK], BF16,
                                   isOutput=False)
    wsh = nc.declare_dram_parameter("wsh", [O_SH, D_IN], BF16, isOutput=False)
    ssh = nc.declare_dram_parameter("ssh", [O_SH, D_IN], F32, isOutput=False)
    nsh = nc.declare_dram_parameter("nsh", [O_SH, D_IN], F32, isOutput=False)
    bsh = nc.declare_dram_parameter("bsh", [1, O_SH], F32, isOutput=False)
    out = nc.declare_dram_parameter("out", [S_SH, O_SH], BF16, isOutput=True)

    xd_r = xd.rearrange("(b p) (k s) -> b p k s", p=P, s=S_BLK)
    ssh_r = ssh.rearrange("o (m g) -> o m g", m=4)
    nsh_r = nsh.rearrange("o (m g) -> o m g", m=4)
    wsh_r = wsh.rearrange("o (m g) -> o m g", m=4)

    with tile.TileContext(nc) as tc:
      with tc.tile_pool(name="const", bufs=1) as const:
        # --- persistent tiles ---
        # Masked weight, transposed: wmt[p, k, o] = Wm'[o, 128k+p] (d' order)
        wmt = const.tile([P, K_TILES, O_SH], BF16, tag="wmt")
        ones1 = const.tile([1, P], F32, tag="ones1")
        nc.any.memset(ones1[:], 1.0)
        bias_f32 = const.tile([1, O_SH], F32, tag="bias_f32")
        nc.sync.dma_start(out=bias_f32[:], in_=bsh[:, :])
        biasB = const.tile([P, O_SH], BF16, tag="biasB")
        epsb = const.tile([P, 1], F32, tag="epsb")
        nc.any.memset(epsb[:], EPS)

        with (
            tc.tile_pool(name="p1io", bufs=2) as p1io,
            tc.tile_pool(name="p1t", bufs=2) as p1t,
            tc.tile_pool(name="p1c", bufs=2) as p1c,
            tc.tile_pool(name="xb", bufs=2) as xbp,
            tc.tile_pool(name="osb", bufs=4) as osbp,
            tc.tile_pool(name="psA", bufs=4, space="PSUM") as psA,
            tc.tile_pool(name="psB", bufs=4, space="PSUM") as psB,
        ):
            # broadcast bias to all 128 partitions via two K=1 fp32 matmuls
            # (one PSUM bank is 512 fp32, bias row is 1024 wide)
            for i in range(2):
                bps = psA.tile([P, OH], F32, tag="ps")
                nc.tensor.matmul(bps[:], ones1[:],
                                 bias_f32[:, i * OH:(i + 1) * OH],
                                 start=True, stop=True)
                nc.vector.tensor_copy(biasB[:, i * OH:(i + 1) * OH], bps[:])

            # --- phase 1: mask generation + masked weight (transposed) ---
            n_gc = G // GC
            for ot in range(O_TILES):
                o0 = ot * P
                for h in range(n_gc):
                    g0 = h * GC
                    sc = p1io.tile([P, 4, GC], F32, tag="sc")
                    nu = p1io.tile([P, 4, GC], F32, tag="nu")
                    w = p1io.tile([P, 4, GC], BF16, tag="w")
                    nc.scalar.dma_start(out=sc[:], in_=ssh_r[o0:o0 + P, :, g0:g0 + GC])
                    nc.scalar.dma_start(out=nu[:], in_=nsh_r[o0:o0 + P, :, g0:g0 + GC])
                    nc.scalar.dma_start(out=w[:], in_=wsh_r[o0:o0 + P, :, g0:g0 + GC])

                    wmb = p1t.tile([P, 4, GC], BF16, tag="wmb")
                    # gumbel chain, mirroring jax fp32 op order (in-place):
                    # nu <- ln(u + eps); nu <- ln(-nu + eps); sc <- sc - nu
                    nc.scalar.activation(nu[:], nu[:], AF.Ln, bias=epsb[:])
                    nc.scalar.activation(nu[:], nu[:], AF.Ln, bias=epsb[:], scale=-1.0)
                    nc.vector.tensor_sub(sc[:], sc[:], nu[:])

                    yk = [sc[:, k, :] for k in range(4)]

                    def cmp(a, b):
                        # bf16 result (exact 0/1) so combine ops run 2x
                        t = p1c.tile([P, GC], BF16, tag=f"ge{a}{b}")
                        nc.vector.tensor_tensor(t[:], yk[a][:], yk[b][:], ALU.is_ge)
                        return t

                    ge01, ge02, ge03 = cmp(0, 1), cmp(0, 2), cmp(0, 3)
                    ge12, ge13, ge23 = cmp(1, 2), cmp(1, 3), cmp(2, 3)

                    def keep_apply(k, terms, thr, op):
                        # sum(terms) (with signs) `op` thr -> *w_k -> wm_k
                        a = p1c.tile([P, GC], BF16, tag="acc0")
                        s = p1c.tile([P, GC], BF16, tag="acc1")
                        nc.vector.tensor_tensor(a[:], terms[0][0][:], terms[1][0][:],
                                                ALU.add if terms[1][1] > 0 else ALU.subtract)
                        nc.vector.tensor_tensor(s[:], a[:], terms[2][0][:],
                                                ALU.add if terms[2][1] > 0 else ALU.subtract)
                        nc.vector.scalar_tensor_tensor(
                            wmb[:, k, :], s[:], float(thr), w[:, k, :],
                            op, ALU.mult)

                    # keep_0: ge01+ge02+ge03 >= 2  (thr 1.5, is_ge)
                    keep_apply(0, [(ge01, 1), (ge02, 1), (ge03, 1)], 1.5, ALU.is_ge)
                    # keep_1: ge12+ge13-ge01 >= 1  (thr 0.5, is_ge)
                    keep_apply(1, [(ge12, 1), (ge13, 1), (ge01, -1)], 0.5, ALU.is_ge)
                    # keep_2: ge23-ge02-ge12 >= 0  (thr -0.5, is_ge)
                    keep_apply(2, [(ge23, 1), (ge02, -1), (ge12, -1)], -0.5, ALU.is_ge)
                    # keep_3: ge03+ge13+ge23 <= 1  (thr 1.5, is_le)
                    keep_apply(3, [(ge03, 1), (ge13, 1), (ge23, 1)], 1.5, ALU.is_le)

                    # transpose masked weight into wmt[p, k', o-block];
                    # tile (m, j) covers d' = m*1024 + g0 + j*128
                    for m in range(4):
                        for j in range(GC // P):
                            kp = m * (G // P) + (g0 // P) + j
                            nc.sync.dma_start_transpose(
                                out=wmt[:, kp, o0:o0 + P],
                                in_=wmb[:, m, j * P:(j + 1) * P])

            # --- phase 2: out[s_blk, :] = x[s_blk, :] @ Wm^T + bias ---
            # o-halves run as separate N=512 chains so the first half's
            # matmuls only depend on phase-1 ot=0..3 (earlier PE start);
            # each half has its own PSUM pool so stalled half-1 chains
            # don't block half-0 chains in the buffer ring.
            for blk in range(N_BLK):
                s0 = blk * S_BLK
                xb = xbp.tile([P, K_TILES, S_BLK], BF16, tag="xb")
                nc.gpsimd.dma_start(out=xb[:], in_=xd_r[blk])
                for st in range(S_BLK // P):
                    for half, pool in ((0, psA), (1, psB)):
                        oh0 = half * OH
                        psum = pool.tile([P, OH], F32, tag="ps")
                        for k in range(K_TILES):
                            nc.tensor.matmul(
                                psum[:],
                                xb[:, k, st * P:(st + 1) * P],
                                wmt[:, k, oh0:oh0 + OH],
                                start=(k == 0), stop=(k == K_TILES - 1))
                        o_sb = osbp.tile([P, OH], BF16, tag=f"osb{half}")
                        nc.vector.tensor_tensor(
                            o_sb[:], psum[:], biasB[:, oh0:oh0 + OH],
                            ALU.add)
                        nc.sync.dma_start(
                            out=out[s0 + st * P: s0 + (st + 1) * P,
                                    oh0:oh0 + OH],
                            in_=o_sb[:])
    nc.compile()
    return nc


def _get_nc():
    if "nc" not in _CACHED:
        _CACHED["nc"] = _build_nc()
    return _CACHED["nc"]


def _dperm_cols(a):
    """Permute the last dim from d = 4g+m order to d' = m*1024+g order."""
    s = a.shape
    return np.ascontiguousarray(
        a.reshape(s[:-1] + (G, 4)).swapaxes(-1, -2).reshape(s[:-1] + (D_IN,)))


def make_in_maps(x, weight, bias, scores, noise_u):
    x = np.asarray(x, dtype=np.float32).reshape(S_TOT, D_IN)
    weight = np.asarray(weight, dtype=np.float32)
    bias = np.asarray(bias, dtype=np.float32)
    scores = np.asarray(scores, dtype=np.float32).reshape(D_OUT, D_IN)
    noise_u = np.asarray(noise_u, dtype=np.float32).reshape(D_OUT, D_IN)

    # x: bf16, d-permuted, blocked [N_BLK, P, K_TILES, S_BLK] per s-half
    xh = _dperm_cols(x.astype(BF16_NP))
    xs = []
    for i in range(S_WAYS):
        xi = xh[i * S_SH:(i + 1) * S_SH]
        xi = xi.reshape(N_BLK, S_BLK, K_TILES, P).transpose(0, 3, 2, 1)
        xs.append(np.ascontiguousarray(xi).reshape(N_BLK * P, K_TILES * S_BLK))

    wp = _dperm_cols(weight.astype(BF16_NP))
    sp = _dperm_cols(scores)
    npm = _dperm_cols(noise_u)

    in_maps = []
    for j in range(N_CORES):
        si, oj = j // O_WAYS, j % O_WAYS
        o0 = oj * O_SH
        in_maps.append({
            "xd": xs[si],
            "wsh": np.ascontiguousarray(wp[o0:o0 + O_SH]),
            "ssh": np.ascontiguousarray(sp[o0:o0 + O_SH]),
            "nsh": np.ascontiguousarray(npm[o0:o0 + O_SH]),
            "bsh": np.ascontiguousarray(bias[o0:o0 + O_SH]).reshape(1, O_SH),
        })
    return in_maps


def kernel(x, weight, bias, scores, noise_u):
    global LAST_EXEC_NS
    in_maps = make_in_maps(x, weight, bias, scores, noise_u)
    nc = _get_nc()
    if os.environ.get("BASS_KERNEL_TIMED", "0") == "1":
        results, exec_ns = _run_timed(nc, in_maps)
        LAST_EXEC_NS = exec_ns
    else:
        res = run_bass_kernel_spmd(nc, in_maps, list(range(N_CORES)), trace=False)
        LAST_EXEC_NS = res.exec_time_ns
        results = res.results
    full = np.empty((S_TOT, D_OUT), dtype=np.float32)
    for j in range(N_CORES):
        si, oj = j // O_WAYS, j % O_WAYS
        full[si * S_SH:(si + 1) * S_SH, oj * O_SH:(oj + 1) * O_SH] = \
            np.asarray(results[j]["out"]).astype(np.float32)
    return full.reshape(B, S, D_OUT)


def _run_timed(nc, in_maps, n_iters=64):
    """Mimic bass2jax.run_bass_via_pjrt multi-core path, but keep inputs
    device-resident and time pipelined repeat executions."""
    import time
    import jax
    from jax.sharding import Mesh, PartitionSpec, NamedSharding
    from jax.experimental.shard_map import shard_map
    from concourse import bass2jax, mybir as _mb

    bass2jax.install_neuronx_cc_hook()
    n_cores = len(in_maps)
    partition_name = (nc.partition_id_tensor.name
                      if nc.partition_id_tensor else None)
    in_names, out_names, out_avals = [], [], []
    for alloc in nc.m.functions[0].allocations:
        if not isinstance(alloc, _mb.MemoryLocationSet):
            continue
        name = alloc.memorylocations[0].name
        if alloc.kind == "ExternalInput":
            if name != partition_name:
                in_names.append(name)
        elif alloc.kind == "ExternalOutput":
            out_names.append(name)
            out_avals.append(jax.core.ShapedArray(
                tuple(alloc.tensor_shape), _mb.dt.np(alloc.dtype)))
    n_params = len(in_names)
    all_names = in_names + out_names + ([partition_name] if partition_name else [])

    def _body(*args):
        operands = list(args)
        if partition_name is not None:
            operands.append(bass2jax.partition_id_tensor())
        return tuple(bass2jax._bass_exec_p.bind(
            *operands, out_avals=tuple(out_avals), in_names=tuple(all_names),
            out_names=tuple(out_names), lowering_input_output_aliases=(),
            sim_require_finite=True, sim_require_nnan=True, nc=nc))

    devices = jax.devices()[:n_cores]
    mesh = Mesh(np.array(devices), ("core",))
    spec = PartitionSpec("core")
    n_outs = len(out_names)
    fn = jax.jit(shard_map(_body, mesh=mesh,
                           in_specs=(spec,) * (n_params + n_outs),
                           out_specs=(spec,) * n_outs, check_rep=False),
                 keep_unused=True)
    sh = NamedSharding(mesh, spec)
    ins_dev = [jax.device_put(
        np.concatenate([np.asarray(m[nm]) for m in in_maps], axis=0), sh)
        for nm in in_names]
    zeros_dev = [jax.device_put(
        np.zeros((n_cores * a.shape[0], *a.shape[1:]), a.dtype), sh)
        for a in out_avals]
    outs = fn(*ins_dev, *zeros_dev)     # compile + warm
    jax.block_until_ready(outs)

    def timed_batch(depth):
        t0 = time.perf_counter()
        for _ in range(depth):
            r = fn(*ins_dev, *zeros_dev)  # pipelined async dispatch
        t_enq = time.perf_counter() - t0
        jax.block_until_ready(r)
        t_tot = time.perf_counter() - t0
        print(f"[kernel]   depth {depth}: enqueue {t_enq*1e3:.1f} ms, "
              f"total {t_tot*1e3:.1f} ms ({t_tot/depth*1e3:.3f} ms/call)",
              flush=True)
        return t_tot / depth, r

    n_iters = int(os.environ.get("BASS_TIMED_ITERS", n_iters))
    d1, d2 = max(8, n_iters // 4), n_iters
    t1, _ = timed_batch(d1)
    t2, last = timed_batch(d2)
    # model t(d) = L/d + T: amortized per-call latency L, true throughput T
    T = (d2 * t2 - d1 * t1) / (d2 - d1)
    print(f"[kernel] pipelined per-call: depth {d1}: {t1*1e3:.2f} ms, "
          f"depth {d2}: {t2*1e3:.2f} ms -> fitted throughput {T*1e3:.3f} ms",
          flush=True)
    dt_ns = min(t2, max(T, 0.0) or t2) * 1e9
    results = [
        {nm: np.asarray(last[i]).reshape(n_cores, *out_avals[i].shape)[c]
         for i, nm in enumerate(out_names)}
        for c in range(n_cores)]
    return results, int(dt_ns)


# revision 12
# speedup vs baseline: 1.1503x; 1.1503x over previous
"""Trainium2 Bass kernel for HDSLinear (gumbel top-2-of-4 masked linear).

Strategy (column-parallel, per sharding hint):
  - Shard weight/scores/noise_u/bias along out_features across 8 cores
    (512 rows each); replicate x.
  - Host-side relayouts (pure permutation/cast, no arithmetic):
      * x is cast to bf16 and stored fully blocked as
        [N_BLK, 128, K_TILES, S_BLK] so each x stage-in is ONE contiguous
        HBM read at line rate (the fp32 strided gather was the baseline's
        bottleneck).
      * the contraction dim d is globally permuted d' = (m, g) (group
        member index outermost) on BOTH x and weight/scores/noise, so all
        group-of-4 compare/select ops on device are stride-1.
      * weight is pre-cast to bf16 (it is multiplied in bf16 anyway).
  - Each core computes its mask shard from scores+gumbel noise on device
    (ACT: 2x Ln; DVE: pairwise-compare rank select, bf16 combine ops),
    applies it to the weight shard, transposes the masked weight on-chip
    (xbar DMA transpose, bf16) into wmt[p, k, o].
  - Phase 2 runs x @ Wm^T as PE matmul chains (32 k-tiles into one PSUM
    bank per 128-row s-tile), adds bias via a DVE add against a
    broadcast bias tile (built once with a K=1 matmul), and streams out
    bf16 [16384, 512] per core.
  - Host concatenates the 8 output shards along out_features, upcasts.
"""

import os
import sys
import numpy as np

for _p in ("/opt/trn_rl_repo", "/root/.axon_site/_ro/trn_rl_repo"):
    if os.path.isdir(_p) and _p not in sys.path:
        sys.path.insert(0, _p)

import concourse.bass as bass
import concourse.bacc as bacc
import concourse.mybir as mybir
from concourse import tile
from concourse.bass_utils import run_bass_kernel_spmd

F32 = mybir.dt.float32
BF16 = mybir.dt.bfloat16
BF16_NP = mybir.dt.np(BF16)
AF = mybir.ActivationFunctionType
ALU = mybir.AluOpType

B, S, D_IN, D_OUT = 8, 2048, 4096, 4096
N_CORES = 8
S_TOT = B * S                      # 16384
O_SH = D_OUT // N_CORES            # 512 out-features per core
P = 128
EPS = 1e-10

K_TILES = D_IN // P                # 32 contraction tiles
S_BLK = 512                        # s-columns per phase-2 block
N_BLK = S_TOT // S_BLK             # 32 blocks
O_TILES = O_SH // P                # 4 o-tiles of 128 rows in phase 1
GC = 512                           # groups per phase-1 tile chunk
G = D_IN // 4                      # 1024 groups per row

LAST_EXEC_NS = None
_CACHED = {}


def _build_nc():
    nc = bacc.Bacc(None, target_bir_lowering=False)
    xd = nc.declare_dram_parameter("xd", [N_BLK * P, K_TILES * S_BLK], BF16,
                                   isOutput=False)
    wsh = nc.declare_dram_parameter("wsh", [O_SH, D_IN], BF16, isOutput=False)
    ssh = nc.declare_dram_parameter("ssh", [O_SH, D_IN], F32, isOutput=False)
    nsh = nc.declare_dram_parameter("nsh", [O_SH, D_IN], F32, isOutput=False)
    bsh = nc.declare_dram_parameter("bsh", [1, O_SH], F32, isOutput=False)
    out = nc.declare_dram_parameter("out", [S_TOT, O_SH], BF16, isOutput=True)

    xd_r = xd.rearrange("(b p) (k s) -> b p k s", p=P, s=S_BLK)
    ssh_r = ssh.rearrange("o (m g) -> o m g", m=4)
    nsh_r = nsh.rearrange("o (m g) -> o m g", m=4)
    wsh_r = wsh.rearrange("o (m g) -> o m g", m=4)

    with tile.TileContext(nc) as tc:
      with tc.tile_pool(name="const", bufs=1) as const:
        # --- persistent tiles ---
        # Masked weight, transposed: wmt[p, k, o] = Wm'[o, 128k+p] (d' order)
        wmt = const.tile([P, K_TILES, O_SH], BF16, tag="wmt")
        ones1 = const.tile([1, P], F32, tag="ones1")
        nc.any.memset(ones1[:], 1.0)
        bias_f32 = const.tile([1, O_SH], F32, tag="bias_f32")
        nc.sync.dma_start(out=bias_f32[:], in_=bsh[:, :])
        biasB = const.tile([P, O_SH], F32, tag="biasB")
        epsb = const.tile([P, 1], F32, tag="epsb")
        nc.any.memset(epsb[:], EPS)

        with (
            tc.tile_pool(name="p1io", bufs=2) as p1io,
            tc.tile_pool(name="p1t", bufs=2) as p1t,
            tc.tile_pool(name="p1c", bufs=2) as p1c,
            tc.tile_pool(name="xb", bufs=2) as xbp,
            tc.tile_pool(name="osb", bufs=4) as osbp,
            tc.tile_pool(name="ps", bufs=8, space="PSUM") as ps,
        ):
            # broadcast bias to all 128 partitions via a K=1 fp32 matmul
            bps = ps.tile([P, O_SH], F32, tag="ps")
            nc.tensor.matmul(bps[:], ones1[:], bias_f32[:], start=True, stop=True)
            nc.scalar.copy(biasB[:], bps[:])

            # --- phase 1: mask generation + masked weight (transposed) ---
            n_gc = G // GC
            for ot in range(O_TILES):
                o0 = ot * P
                for h in range(n_gc):
                    g0 = h * GC
                    sc = p1io.tile([P, 4, GC], F32, tag="sc")
                    nu = p1io.tile([P, 4, GC], F32, tag="nu")
                    w = p1io.tile([P, 4, GC], BF16, tag="w")
                    nc.scalar.dma_start(out=sc[:], in_=ssh_r[o0:o0 + P, :, g0:g0 + GC])
                    nc.scalar.dma_start(out=nu[:], in_=nsh_r[o0:o0 + P, :, g0:g0 + GC])
                    nc.scalar.dma_start(out=w[:], in_=wsh_r[o0:o0 + P, :, g0:g0 + GC])

                    wmb = p1t.tile([P, 4, GC], BF16, tag="wmb")
                    # gumbel chain, mirroring jax fp32 op order (in-place):
                    # nu <- ln(u + eps); nu <- ln(-nu + eps); sc <- sc - nu
                    nc.scalar.activation(nu[:], nu[:], AF.Ln, bias=epsb[:])
                    nc.scalar.activation(nu[:], nu[:], AF.Ln, bias=epsb[:], scale=-1.0)
                    nc.vector.tensor_sub(sc[:], sc[:], nu[:])

                    yk = [sc[:, k, :] for k in range(4)]

                    def cmp(a, b):
                        # bf16 result (exact 0/1) so combine ops run 2x
                        t = p1c.tile([P, GC], BF16, tag=f"ge{a}{b}")
                        nc.vector.tensor_tensor(t[:], yk[a][:], yk[b][:], ALU.is_ge)
                        return t

                    ge01, ge02, ge03 = cmp(0, 1), cmp(0, 2), cmp(0, 3)
                    ge12, ge13, ge23 = cmp(1, 2), cmp(1, 3), cmp(2, 3)

                    def keep_apply(k, terms, thr, op):
                        # sum(terms) (with signs) `op` thr -> *w_k -> wm_k
                        a = p1c.tile([P, GC], BF16, tag="acc0")
                        s = p1c.tile([P, GC], BF16, tag="acc1")
                        nc.vector.tensor_tensor(a[:], terms[0][0][:], terms[1][0][:],
                                                ALU.add if terms[1][1] > 0 else ALU.subtract)
                        nc.vector.tensor_tensor(s[:], a[:], terms[2][0][:],
                                                ALU.add if terms[2][1] > 0 else ALU.subtract)
                        nc.vector.scalar_tensor_tensor(
                            wmb[:, k, :], s[:], float(thr), w[:, k, :],
                            op, ALU.mult)

                    # keep_0: ge01+ge02+ge03 >= 2  (thr 1.5, is_ge)
                    keep_apply(0, [(ge01, 1), (ge02, 1), (ge03, 1)], 1.5, ALU.is_ge)
                    # keep_1: ge12+ge13-ge01 >= 1  (thr 0.5, is_ge)
                    keep_apply(1, [(ge12, 1), (ge13, 1), (ge01, -1)], 0.5, ALU.is_ge)
                    # keep_2: ge23-ge02-ge12 >= 0  (thr -0.5, is_ge)
                    keep_apply(2, [(ge23, 1), (ge02, -1), (ge12, -1)], -0.5, ALU.is_ge)
                    # keep_3: ge03+ge13+ge23 <= 1  (thr 1.5, is_le)
                    keep_apply(3, [(ge03, 1), (ge13, 1), (ge23, 1)], 1.5, ALU.is_le)

                    # transpose masked weight into wmt[p, k', o-block];
                    # tile (m, j) covers d' = m*1024 + g0 + j*128
                    for m in range(4):
                        for j in range(GC // P):
                            kp = m * (G // P) + (g0 // P) + j
                            nc.sync.dma_start_transpose(
                                out=wmt[:, kp, o0:o0 + P],
                                in_=wmb[:, m, j * P:(j + 1) * P])

            # --- phase 2: out[s_blk, :] = x[s_blk, :] @ Wm^T + bias ---
            for blk in range(N_BLK):
                s0 = blk * S_BLK
                xb = xbp.tile([P, K_TILES, S_BLK], BF16, tag="xb")
                nc.sync.dma_start(out=xb[:], in_=xd_r[blk])
                for st in range(S_BLK // P):
                    psum = ps.tile([P, O_SH], F32, tag="ps")
                    for k in range(K_TILES):
                        nc.tensor.matmul(
                            psum[:],
                            xb[:, k, st * P:(st + 1) * P],
                            wmt[:, k, :],
                            start=(k == 0), stop=(k == K_TILES - 1))
                    o_sb = osbp.tile([P, O_SH], BF16, tag="osb")
                    nc.vector.tensor_tensor(o_sb[:], psum[:], biasB[:], ALU.add)
                    nc.scalar.dma_start(
                        out=out[s0 + st * P: s0 + (st + 1) * P, :],
                        in_=o_sb[:])
    nc.compile()
    return nc


def _get_nc():
    if "nc" not in _CACHED:
        _CACHED["nc"] = _build_nc()
    return _CACHED["nc"]


def _dperm_cols(a):
    """Permute the last dim from d = 4g+m order to d' = m*1024+g order."""
    s = a.shape
    return np.ascontiguousarray(
        a.reshape(s[:-1] + (G, 4)).swapaxes(-1, -2).reshape(s[:-1] + (D_IN,)))


def make_in_maps(x, weight, bias, scores, noise_u):
    x = np.asarray(x, dtype=np.float32).reshape(S_TOT, D_IN)
    weight = np.asarray(weight, dtype=np.float32)
    bias = np.asarray(bias, dtype=np.float32)
    scores = np.asarray(scores, dtype=np.float32).reshape(D_OUT, D_IN)
    noise_u = np.asarray(noise_u, dtype=np.float32).reshape(D_OUT, D_IN)

    # x: bf16, d-permuted, fully blocked [N_BLK, P, K_TILES, S_BLK]
    xh = _dperm_cols(x.astype(BF16_NP))
    xh = xh.reshape(N_BLK, S_BLK, K_TILES, P).transpose(0, 3, 2, 1)
    xh = np.ascontiguousarray(xh).reshape(N_BLK * P, K_TILES * S_BLK)

    wp = _dperm_cols(weight.astype(BF16_NP))
    sp = _dperm_cols(scores)
    npm = _dperm_cols(noise_u)

    in_maps = []
    for j in range(N_CORES):
        o0 = j * O_SH
        in_maps.append({
            "xd": xh,
            "wsh": np.ascontiguousarray(wp[o0:o0 + O_SH]),
            "ssh": np.ascontiguousarray(sp[o0:o0 + O_SH]),
            "nsh": np.ascontiguousarray(npm[o0:o0 + O_SH]),
            "bsh": np.ascontiguousarray(bias[o0:o0 + O_SH]).reshape(1, O_SH),
        })
    return in_maps


def kernel(x, weight, bias, scores, noise_u):
    global LAST_EXEC_NS
    in_maps = make_in_maps(x, weight, bias, scores, noise_u)
    nc = _get_nc()
    if os.environ.get("BASS_KERNEL_TIMED", "0") == "1":
        results, exec_ns = _run_timed(nc, in_maps)
        LAST_EXEC_NS = exec_ns
    else:
        res = run_bass_kernel_spmd(nc, in_maps, list(range(N_CORES)), trace=False)
        LAST_EXEC_NS = res.exec_time_ns
        results = res.results
    out = np.concatenate(
        [np.asarray(results[j]["out"]).astype(np.float32)
         for j in range(N_CORES)], axis=1)
    return out.reshape(B, S, D_OUT)


def _run_timed(nc, in_maps, n_iters=1024):
    """Mimic bass2jax.run_bass_via_pjrt multi-core path, but keep inputs
    device-resident and time pipelined repeat executions."""
    import time
    import jax
    from jax.sharding import Mesh, PartitionSpec, NamedSharding
    from jax.experimental.shard_map import shard_map
    from concourse import bass2jax, mybir as _mb

    bass2jax.install_neuronx_cc_hook()
    n_cores = len(in_maps)
    partition_name = (nc.partition_id_tensor.name
                      if nc.partition_id_tensor else None)
    in_names, out_names, out_avals = [], [], []
    for alloc in nc.m.functions[0].allocations:
        if not isinstance(alloc, _mb.MemoryLocationSet):
            continue
        name = alloc.memorylocations[0].name
        if alloc.kind == "ExternalInput":
            if name != partition_name:
                in_names.append(name)
        elif alloc.kind == "ExternalOutput":
            out_names.append(name)
            out_avals.append(jax.core.ShapedArray(
                tuple(alloc.tensor_shape), _mb.dt.np(alloc.dtype)))
    n_params = len(in_names)
    all_names = in_names + out_names + ([partition_name] if partition_name else [])

    def _body(*args):
        operands = list(args)
        if partition_name is not None:
            operands.append(bass2jax.partition_id_tensor())
        return tuple(bass2jax._bass_exec_p.bind(
            *operands, out_avals=tuple(out_avals), in_names=tuple(all_names),
            out_names=tuple(out_names), lowering_input_output_aliases=(),
            sim_require_finite=True, sim_require_nnan=True, nc=nc))

    devices = jax.devices()[:n_cores]
    mesh = Mesh(np.array(devices), ("core",))
    spec = PartitionSpec("core")
    n_outs = len(out_names)
    fn = jax.jit(shard_map(_body, mesh=mesh,
                           in_specs=(spec,) * (n_params + n_outs),
                           out_specs=(spec,) * n_outs, check_rep=False),
                 keep_unused=True)
    sh = NamedSharding(mesh, spec)
    ins_dev = [jax.device_put(
        np.concatenate([np.asarray(m[nm]) for m in in_maps], axis=0), sh)
        for nm in in_names]
    zeros_dev = [jax.device_put(
        np.zeros((n_cores * a.shape[0], *a.shape[1:]), a.dtype), sh)
        for a in out_avals]
    outs = fn(*ins_dev, *zeros_dev)     # compile + warm
    jax.block_until_ready(outs)

    def timed_batch(depth):
        t0 = time.perf_counter()
        for _ in range(depth):
            r = fn(*ins_dev, *zeros_dev)  # pipelined async dispatch
        t_enq = time.perf_counter() - t0
        jax.block_until_ready(r)
        t_tot = time.perf_counter() - t0
        print(f"[kernel]   depth {depth}: enqueue {t_enq*1e3:.1f} ms, "
              f"total {t_tot*1e3:.1f} ms ({t_tot/depth*1e3:.3f} ms/call)",
              flush=True)
        return t_tot / depth, r

    n_iters = int(os.environ.get("BASS_TIMED_ITERS", n_iters))
    d1, d2 = max(8, n_iters // 4), n_iters
    t1, _ = timed_batch(d1)
    t2, last = timed_batch(d2)
    # model t(d) = L/d + T: amortized per-call latency L, true throughput T
    T = (d2 * t2 - d1 * t1) / (d2 - d1)
    print(f"[kernel] pipelined per-call: depth {d1}: {t1*1e3:.2f} ms, "
          f"depth {d2}: {t2*1e3:.2f} ms -> fitted throughput {T*1e3:.3f} ms",
          flush=True)
    dt_ns = min(t2, max(T, 0.0) or t2) * 1e9
    results = [
        {nm: np.asarray(last[i]).reshape(n_cores, *out_avals[i].shape)[c]
         for i, nm in enumerate(out_names)}
        for c in range(n_cores)]
    return results, int(dt_ns)
